# revision 1
# baseline (speedup 1.0000x reference)
"""Bahdanau-additive attention scorer on 8 TRN2 NeuronCores.

reference:
  wq = context @ Wc.T            (B, CTX, D)
  uh = queries @ Wq.T + bq       (B, QRS, D)
  scores[b,c,q] = sum_h v[h] * tanh(wq[b,c,h] + uh[b,q,h])
  return scores.reshape(B, QRS, CTX)     # flat view of (B, CTX, QRS)

Sharding: over (batch, query): core k handles batch k//4, queries
(k%4)*64 ... +64, with the full 1024-row context of its batch (context
replicated across the 4 cores of a batch, weights everywhere).

Queries and v are pre-split on host into fp16 (hi, lo) pairs; context and
the weight matrices ship as single fp16 (total output error ~4e-4 relative,
~50x under the correctness gate, in exchange for minimal input DMA and
1-cycle/row PE matmuls).

Device layout: hidden dim h on partitions (2 tiles of 128 = "m" halves).
  prologue (PE): wqT[h,c] = Wc @ ctx.T (2 x [128,1024] psum, cast to fp16)
                 uhT[h,q] = Wq @ q.T + bq (2 x [128,64], fp32)
  main loop over groups of units (unit = (m, q), m-major, sizes in
  GROUP_SIZES — small at the edges for pipeline head/tail, 16 in the
  middle to amortize the ~222-cycle ACT per-instruction init):
    group 0 is FUSED on ACT: tanh(wq_psum + uh[q]) with the per-partition
    bias read directly from the prologue's psum — the ACT stream starts
    as soon as the first wq matmuls land, skipping cast+add latency.
    other groups: S = wqT_m + uhT_m[:, q]  DVE tensor_scalar_add (fp16, 4x)
                  T = tanh(S)  fp16        ACT (the bottleneck: ~109us/core)
    reduce: per 128-col chunk: psum[:, col:col+2] = T_chunk.T @ (v_hi|v_lo)
            PE, self-loading fp16 lhsT, N=2, every matmul a closed group
  epilogue (overlapped): DVE sums the 4 psum planes (m x hi/lo) per chunk
  as soon as its q-range completes, chunked DMA to DRAM.

Output per core: [128, 512] = scores[b, csub*128 + p, q0 + col//8],
csub = col % 8; host reassembles + final reshape.
"""

import numpy as np

import concourse.bacc as bacc
import concourse.mybir as mybir
import concourse.tile as tile
from concourse.bass_utils import run_bass_kernel_spmd

F32 = mybir.dt.float32
F16 = mybir.dt.float16
TANH = mybir.ActivationFunctionType.Tanh
ADD = mybir.AluOpType.add

B, CTX, QRS, D = 2, 1024, 256, 256
N_CORES = 8
QL = (B * QRS) // N_CORES        # 64 queries per core
UNITS = 2 * QL                   # (m, q) pairs
# group sizes (units per S/T tile): small edge groups tighten the
# pipeline head/tail; sum must be UNITS and no group may straddle u=QL
GROUP_SIZES = [4, 4, 8] + [16] * 6 + [8, 4, 2, 2]


def _build_nc():
    nc = bacc.Bacc("TRN2", target_bir_lowering=False, debug=False,
                   enable_asserts=False)

    # host-prepped inputs. DMA dispatch costs ~0.6us per descriptor on an
    # engine sequencer, so inputs are packed into few tensors ordered by
    # when the prologue needs them: wcp = wc fp16, uhp = (wq|q_hi|q_lo),
    # small = (bq2 f32 | v hi/lo fp16 pairs bitcast to f32).
    WCC = D                                          # wc single fp16
    UHC = D + 2 * QL                                 # wq single + q hi/lo
    wcp_d = nc.dram_tensor("wcp", [D, WCC], F16, kind="ExternalInput")
    uhp_d = nc.dram_tensor("uhp", [D, UHC], F16, kind="ExternalInput")
    # context is single fp16 (no hi/lo): its quantization adds only ~8e-5
    # to the output error but halves the dominant input DMA + wq matmuls
    ctx_d = nc.dram_tensor("ctx", [D, CTX], F16, kind="ExternalInput")
    small_d = nc.dram_tensor("small", [128, 4], F32, kind="ExternalInput")
    out = nc.dram_tensor("out", [128, 8 * QL], F32, kind="ExternalOutput")

    with tile.TileContext(nc) as tc:
        with (
            tc.tile_pool(name="consts", bufs=1) as cp,
            tc.tile_pool(name="sp", bufs=2) as sp,
            tc.tile_pool(name="tp", bufs=2) as tp,
            tc.tile_pool(name="pre_ps", bufs=2, space="PSUM") as ppre,
            tc.tile_pool(name="out_ps", bufs=1, space="PSUM") as pout,
        ):
            # ---------- load inputs: 7 DMAs on the two HWDGE queues ----------
            # (only SP/sync and ACT/scalar have HWDGE rings; SWDGE via gpsimd
            # measured ~6us startup, so it gets nothing). DMA dispatches are
            # emitted before anything else so they head both queues.
            small_t = cp.tile([128, 4], F32, tag="small", name="small_t")
            wcpk = [cp.tile([128, WCC], F16, tag=f"wcp{k}", name=f"wcp{k}")
                    for k in range(2)]
            uhpk = [cp.tile([128, UHC], F16, tag=f"uhp{k}", name=f"uhp{k}")
                    for k in range(2)]
            # ctx as per-(k, n-half) tiles: the n0 halves land ~1.4us
            # earlier than full k-tiles would, unblocking the first wq
            # matmuls and the fused group sooner
            ctxh = [[cp.tile([128, 512], F16, tag=f"ctx{k}{n}",
                             name=f"ctx{k}{n}") for n in range(2)]
                    for k in range(2)]
            nc.sync.dma_start(small_t[:], small_d[:])
            nc.sync.dma_start(wcpk[0][:], wcp_d[0:128, :])
            nc.sync.dma_start(wcpk[1][:], wcp_d[128:256, :])
            nc.scalar.dma_start(uhpk[0][:], uhp_d[0:128, :])
            nc.scalar.dma_start(uhpk[1][:], uhp_d[128:256, :])
            nc.sync.dma_start(ctxh[0][0][:], ctx_d[0:128, 0:512])
            nc.scalar.dma_start(ctxh[1][0][:], ctx_d[128:256, 0:512])
            nc.sync.dma_start(ctxh[0][1][:], ctx_d[0:128, 512:1024])
            nc.scalar.dma_start(ctxh[1][1][:], ctx_d[128:256, 512:1024])

            # ---------- ACT table warmup ----------
            # first Tanh triggers a ~2.7us ACT_TABLE_LOAD; fire it on a dummy
            # tile right after the DMA dispatches so the load overlaps the
            # prologue instead of delaying the first real tanh
            warm = cp.tile([128, 1], F32, tag="warm", name="warm")
            nc.vector.memset(warm[:], 0.0)
            nc.scalar.activation(warm[:], warm[:], TANH)
            # views into the packed tiles
            wc_t = [wcpk[k][:, 0:D] for k in range(2)]
            wq_t = [uhpk[k][:, 0:D] for k in range(2)]
            q_t = [[uhpk[k][:, D + p * QL:D + (p + 1) * QL]
                    for k in range(2)] for p in range(2)]
            bq_t = small_t[:, 0:2]
            vs = small_t[:, 2:4].bitcast(F16)      # [128, 4] fp16

            # ---------- uh = Wq @ qT + bq  (fp32) ----------
            uhT = [cp.tile([128, QL], F32, tag=f"uhT{m}", name=f"uhT{m}")
                   for m in range(2)]
            for m in range(2):
                msl = slice(m * 128, (m + 1) * 128)
                ps_uh = ppre.tile([128, QL], F32, tag="psuh", name=f"psuh{m}")
                first = True
                for xp in range(2):          # (q_hi, q_lo) x wq single
                    for k in range(2):
                        nc.tensor.matmul(ps_uh[:], lhsT=wq_t[k][:, msl],
                                         rhs=q_t[xp][k][:],
                                         start=first,
                                         stop=xp == 1 and k == 1)
                        first = False
                nc.vector.tensor_scalar_add(uhT[m][:], ps_uh[:],
                                            bq_t[:, m:m + 1])

            # ---------- wqT = Wc @ ctxT  (fp16 out for fast DVE adds) ----------
            wqT = [cp.tile([128, CTX], F16, tag=f"wqT{m}", name=f"wqT{m}")
                   for m in range(2)]
            ps_wq0 = []                    # m=0 psums kept for the fused g0
            for m in range(2):
                msl = slice(m * 128, (m + 1) * 128)
                for n in range(2):
                    nsl = slice(n * 512, (n + 1) * 512)
                    ps_wq = ppre.tile([128, 512], F32, tag="pswq",
                                      name=f"pswq{m}_{n}")
                    first = True
                    for k in range(2):          # ctx single x wc single
                        nc.tensor.matmul(
                            ps_wq[:], lhsT=wc_t[k][:, msl],
                            rhs=ctxh[k][n][:],
                            start=first, stop=k == 1)
                        first = False
                    nc.vector.tensor_copy(wqT[m][:, nsl], ps_wq[:])
                    if m == 0:
                        ps_wq0.append(ps_wq)

            # ---------- main loop ----------
            # scoresT psum [128, 2048]: column (q*8+csub)*4 + m*2 + {hi,lo};
            # every matmul its own closed accumulation group (one open group
            # per psum bank is a HW constraint)
            ps_out = pout.tile([128, 4 * 8 * QL], F32, tag="pso", name="pso")
            stage = cp.tile([128, 8 * QL], F32, tag="stage", name="stage")
            pr = ps_out[:].rearrange("p (a b) -> p a b", b=4)
            planes = [pr[:, :, i:i + 1].squeeze(2) for i in range(4)]

            u0 = 0
            for g, gsz in enumerate(GROUP_SIZES):
                t = tp.tile([128, gsz * CTX], F16, tag="t", name=f"t{g}")
                if g == 0:
                    # fused add+tanh on ACT, reading the wq psum directly
                    # (bias = uh column): the ACT stream starts as soon as
                    # the first wq matmuls land, skipping cast+add latency
                    for j in range(gsz):
                        q = u0 + j           # all m=0 units
                        for n in range(2):
                            nc.scalar.activation(
                                t[:, j * CTX + n * 512:j * CTX + n * 512 + 512],
                                ps_wq0[n][:], TANH,
                                bias=uhT[0][:, q:q + 1])
                else:
                    s = sp.tile([128, gsz * CTX], F16, tag="s", name=f"s{g}")
                    for j in range(gsz):
                        u = u0 + j
                        m, q = u // QL, u % QL
                        nc.vector.tensor_scalar_add(
                            s[:, j * CTX:(j + 1) * CTX], wqT[m][:],
                            uhT[m][:, q:q + 1])
                    nc.scalar.activation(t[:], s[:], TANH)
                for j in range(gsz):
                    u = u0 + j
                    m, q = u // QL, u % QL
                    for csub in range(8):
                        col = 4 * (q * 8 + csub) + 2 * m
                        nc.tensor.matmul(
                            ps_out[:, col:col + 2],
                            lhsT=t[:, j * CTX + csub * 128:
                                   j * CTX + (csub + 1) * 128],
                            rhs=vs[:, 2 * m:2 * m + 2],
                            start=True, stop=True)
                # epilogue chunk: once a group of m=1 units is reduced, its
                # q-range has all 4 planes -> combine + store, overlapped
                # with remaining groups (DVE reads at most one PSUM operand
                # per instruction, hence the chain through stage)
                if u0 + gsz > QL:
                    qa = max(u0, QL) - QL
                    qb = u0 + gsz - QL
                    csl = slice(qa * 8, qb * 8)
                    nc.vector.tensor_copy(stage[:, csl], planes[0][:, csl])
                    for i in range(1, 4):
                        nc.vector.scalar_tensor_tensor(
                            stage[:, csl], planes[i][:, csl], 0.0,
                            stage[:, csl], ADD, ADD)
                    nc.sync.dma_start(out[:, csl], stage[:, csl])
                u0 += gsz

    nc.compile()
    return nc


_NC_CACHE = {}


def _get_nc():
    if "nc" not in _NC_CACHE:
        _NC_CACHE["nc"] = _build_nc()
    return _NC_CACHE["nc"]


def _hilo(x):
    hi = x.astype(np.float16)
    lo = (x - hi.astype(np.float32)).astype(np.float16)
    return np.ascontiguousarray(hi), np.ascontiguousarray(lo)


def _in_maps(context, queries, Wc, Wq, bq, v):
    wc_f16 = np.ascontiguousarray(Wc.T.astype(np.float16))
    wq_f16 = np.ascontiguousarray(Wq.T.astype(np.float16))
    bq2 = bq.reshape(2, 128).T.astype(np.float32)  # [128, 2]
    v2 = v.reshape(2, 128).T.astype(np.float32)    # [128, 2]
    vh = v2.astype(np.float16)
    vl = (v2 - vh.astype(np.float32)).astype(np.float16)
    vs = np.stack([vh[:, 0], vl[:, 0], vh[:, 1], vl[:, 1]], axis=1)
    small = np.ascontiguousarray(
        np.concatenate([bq2, vs.view(np.float32)], axis=1))  # [128, 4] f32
    ctx_f16 = [np.ascontiguousarray(context[b].T.astype(np.float16))
               for b in range(B)]
    maps = []
    for k in range(N_CORES):
        b = k // (N_CORES // B)
        q0 = (k % (N_CORES // B)) * QL
        q_p = _hilo(queries[b, q0:q0 + QL, :].T.astype(np.float32))
        wcp = wc_f16
        uhp = np.ascontiguousarray(np.concatenate(
            [wq_f16, q_p[0], q_p[1]], axis=1))
        maps.append({
            "wcp": wcp, "uhp": uhp,
            "ctx": ctx_f16[b],
            "small": small,
        })
    return maps


def run(context, queries, Wc, Wq, bq, v, trace=False, **spmd_kwargs):
    nc = _get_nc()
    maps = _in_maps(np.asarray(context), np.asarray(queries), np.asarray(Wc),
                    np.asarray(Wq), np.asarray(bq), np.asarray(v))
    res = run_bass_kernel_spmd(nc, maps, core_ids=list(range(N_CORES)),
                               trace=trace, **spmd_kwargs)
    scores = np.empty((B, CTX, QRS), dtype=np.float32)
    for k in range(N_CORES):
        b = k // (N_CORES // B)
        q0 = (k % (N_CORES // B)) * QL
        arr = res.results[k]["out"]            # [128, QL*8]
        # arr[p, q*8+csub] = scores[b, csub*128+p, q0+q]
        blk = arr.reshape(128, QL, 8).transpose(2, 0, 1).reshape(CTX, QL)
        scores[b, :, q0:q0 + QL] = blk
    return scores.reshape(B, QRS, CTX), res


def kernel(context, queries, Wc, Wq, bq, v):
    out, _ = run(context, queries, Wc, Wq, bq, v, trace=False)
    return out



# revision 2
# speedup vs baseline: 1.0039x; 1.0039x over previous
"""Bahdanau-additive attention scorer on 8 TRN2 NeuronCores, v2.

Replaces the reference's 134M-element tanh stream (ACT-bound, ~109us/core in
the v1 kernel) with a separable Fourier expansion evaluated on the SMALL
per-side grids plus PE matmuls:

  tanh(z) ~ sum_{j=1..6} w_j sin(om_j z)        (free-freq fit, sup 2.3e-3)
  sin(om(a+b)) = sin(om a)cos(om b) + cos(om a)sin(om b)
  scores[c,q] = sum_e v_e tanh(a[e,c]+b[e,q])
              ~ sum_j  (w_j v . sinA_j)^T cosB_j + (w_j v . cosA_j)^T sinB_j

ACT Sin is only accurate on |arg| <= pi, so for j>=2 the argument is range-
reduced on the DVE in fp16: u = x*om/2pi; t = fp16(u + 1536 + s) rounds to
an integer (fp16 ulp = 1 across [1024,2048), s = 0 for sin / 0.25 for cos);
k = t - (1536+s) (exact quarter-integers); fr = u - k in [-.5,.5] via one
STT (f32 internal math); grid = ACT sin(2pi*fr), bias-free. j=1 is directly
in-domain (om1 capped at 0.30: 0.30*5.05 + pi/2 < pi).

Sharding: core k = (batch k//4, context rows 256*(k%4)..+256) with the full
q=256 of its batch; weights replicated. Device pipeline per core:
  prologue PE: mega-psum [128,1024] = [a fat(e0|e1) | b fat(e0|e1)];
    a = Wc ctxT, b = Wq qT + bq (bias via 1-row matmul against ones).
  per j: DVE wrap chains -> ACT grid instrs (FD=1024, fp16, both e-tiles) ->
    DVE folds of w_j*v into a-halves (2 thin ops per product) ->
    PE 8 matmuls N=256 accumulating into 2 out-psum banks (one open
    accumulation group per bank across all 12 products).
  epilogue: DVE psum->fp16 stage, single DMA out [128,512].

Host side: fp16 transposed/packed inputs; gather 8 tiles -> (B,CTX,QRS) f32
-> reshape(B,QRS,CTX) (flat reinterpretation, faithful to the reference).
"""

import numpy as np

import concourse.bacc as bacc
import concourse.mybir as mybir
import concourse.tile as tile
from concourse.bass_utils import run_bass_kernel_spmd

F32 = mybir.dt.float32
F16 = mybir.dt.float16
SIN = mybir.ActivationFunctionType.Sin
MULT = mybir.AluOpType.mult
SUB = mybir.AluOpType.subtract
ADD = mybir.AluOpType.add

B, CTX, QRS, D = 2, 1024, 256, 256
N_CORES = 8
CL = 256                      # context rows per core
TWO_PI = float(2 * np.pi)

# free-frequency sine fit of tanh on [-8.38, 8.38], om1 capped for the
# unwrapped j=1 path; device-sim output error 0.0064 vs tolerance 0.0526
SINES = [
    (1.22952318, 0.3),
    (0.31341249, 0.9061138),
    (0.11544378, 1.52753214),
    (0.04320977, 2.16783944),
    (0.01782799, 2.82434031),
]


def _build_nc():
    NS = len(SINES)
    NPROD = 2 * NS
    nc = bacc.Bacc("TRN2", target_bir_lowering=False, debug=False,
                   enable_asserts=False)

    ww_d = nc.dram_tensor("ww", [128, 1024], F16, kind="ExternalInput")
    aq_d = nc.dram_tensor("aq", [128, 1024], F16, kind="ExternalInput")
    vm_d = nc.dram_tensor("vm", [128, 2 * NPROD], F32, kind="ExternalInput")
    bqr_d = nc.dram_tensor("bqr", [1, 256], F16, kind="ExternalInput")
    out_d = nc.dram_tensor("out", [128, 512], F16, kind="ExternalOutput")

    with tile.TileContext(nc) as tc:
        with (
            tc.tile_pool(name="consts", bufs=1) as cp,
            tc.tile_pool(name="grids", bufs=1) as gp,
            tc.tile_pool(name="mega", bufs=1, space="PSUM") as mp,
            tc.tile_pool(name="outp", bufs=1, space="PSUM") as op,
        ):
            ww = cp.tile([128, 1024], F16, tag="ww", name="ww")
            aq = cp.tile([128, 1024], F16, tag="aq", name="aq")
            vm = cp.tile([128, 2 * NPROD], F32, tag="vm", name="vm")
            bqr = cp.tile([1, 256], F16, tag="bqr", name="bqr")
            # all input DMAs on the SP queue: it dispatches before the
            # engine-init barrier, so transfers overlap engine startup
            nc.sync.dma_start(ww[:], ww_d[:])
            nc.sync.dma_start(aq[:], aq_d[:])
            nc.sync.dma_start(vm[:], vm_d[:])
            nc.sync.dma_start(bqr[:], bqr_d[:])
            # warm the sin table with a const-AP input (no memset dep)
            warm = cp.tile([128, 1], F32, tag="warm", name="warm")
            nc.scalar.activation(warm[:], nc.const_aps.tensor(0.0, (128, 1), F32),
                                 SIN)
            wcT = ww[:, 0:512]
            wqT = ww[:, 512:1024]
            ctxT = aq[:, 0:512]
            qT = aq[:, 512:1024]

            ones = cp.tile([1, 256], F16, tag="ones", name="ones")
            nc.vector.memset(ones[:], 1.0)
            hpi = cp.tile([128, 1], F32, tag="hpi", name="hpi")
            nc.vector.memset(hpi[:], float(np.pi / 2))

            # ---- prologue: mega = [a(e0)|a(e1)|b(e0)|b(e1)] f32 psum ----
            meg = mp.tile([128, 1024], F32, tag="meg", name="meg")
            for et in range(2):
                sl = slice(et * 256, et * 256 + 256)
                for dt in range(2):
                    nc.tensor.matmul(
                        meg[:, sl],
                        lhsT=wcT[:, dt * 256 + et * 128: dt * 256 + et * 128 + 128],
                        rhs=ctxT[:, dt * 256:(dt + 1) * 256],
                        start=dt == 0, stop=dt == 1)
            for et in range(2):
                sl = slice(512 + et * 256, 512 + et * 256 + 256)
                for dt in range(2):
                    nc.tensor.matmul(
                        meg[:, sl],
                        lhsT=wqT[:, dt * 256 + et * 128: dt * 256 + et * 128 + 128],
                        rhs=qT[:, dt * 256:(dt + 1) * 256],
                        start=dt == 0, stop=False)
                nc.tensor.matmul(
                    meg[:, sl],
                    lhsT=bqr[0:1, et * 128:(et + 1) * 128],
                    rhs=ones[0:1, :],
                    start=False, stop=True)

            # fp16 mega for the wrap chains
            meg16 = cp.tile([128, 1024], F16, tag="meg16", name="meg16")
            nc.scalar.copy(meg16[:], meg[:])

            ops = [op.tile([128, 512], F32, tag=f"ops{ct}", name=f"ops{ct}")
                   for ct in range(2)]

            products = []          # (folded_a_tile, grid_tile_with_b_half)
            for j, (w, om) in enumerate(SINES):
                gs = gp.tile([128, 1024], F16, tag=f"gs{j}", name=f"gs{j}")
                gc = gp.tile([128, 1024], F16, tag=f"gc{j}", name=f"gc{j}")
                if j == 0:
                    nc.scalar.activation(gs[:], meg[:], SIN, scale=float(om))
                    nc.scalar.activation(gc[:], meg[:], SIN, scale=float(om),
                                         bias=hpi[:, 0:1])
                else:
                    sc = float(om / TWO_PI)
                    u = gp.tile([128, 1024], F16, tag=f"u{j}", name=f"u{j}")
                    nc.vector.tensor_scalar_mul(u[:], meg16[:], sc)
                    for g, shift in ((gs, 0.0), (gc, 0.25)):
                        sfx = f"{j}_{int(shift * 4)}"
                        t = gp.tile([128, 1024], F16, tag=f"t{sfx}",
                                    name=f"t{sfx}")
                        k = gp.tile([128, 1024], F16, tag=f"k{sfx}",
                                    name=f"k{sfx}")
                        fr = gp.tile([128, 1024], F16, tag=f"fr{sfx}",
                                     name=f"fr{sfx}")
                        nc.vector.tensor_scalar(t[:], u[:], 1536.0 + shift,
                                                None, ADD)
                        nc.vector.tensor_scalar(k[:], t[:], 1536.0 + shift,
                                                None, SUB)
                        nc.vector.tensor_sub(fr[:], u[:], k[:])
                        nc.scalar.activation(g[:], fr[:], SIN, scale=TWO_PI)
                # fold w_j * v into a-halves
                pidx = 2 * j
                folded = []
                for g, f_name, col in ((gs, f"fs{j}", pidx),
                                       (gc, f"fc{j}", pidx + 1)):
                    f = gp.tile([128, 512], F16, tag=f_name, name=f_name)
                    for et in range(2):
                        nc.vector.tensor_scalar_mul(
                            f[:, et * 256:(et + 1) * 256],
                            g[:, et * 256:(et + 1) * 256],
                            vm[:, 2 * col + et: 2 * col + et + 1])
                    folded.append(f)
                products.append((folded[0], gc))   # sinA x cosB
                products.append((folded[1], gs))   # cosA x sinB

                # matmuls for this j (both products, both c-tiles)
                first = j == 0
                last = j == NS - 1
                for ct in range(2):
                    for pi_, (fa, gb) in enumerate(products[-2:]):
                        for et in range(2):
                            nc.tensor.matmul(
                                ops[ct][:, 0:256],
                                lhsT=fa[:, et * 256 + ct * 128:
                                        et * 256 + ct * 128 + 128],
                                rhs=gb[:, 512 + et * 256: 512 + (et + 1) * 256],
                                start=(first and pi_ == 0 and et == 0),
                                stop=(last and pi_ == 1 and et == 1))

            # ---- epilogue ----
            stage = cp.tile([128, 512], F16, tag="stage", name="stage")
            for ct in range(2):
                nc.scalar.copy(stage[:, ct * 256:(ct + 1) * 256],
                               ops[ct][:, 0:256])
            nc.sync.dma_start(out_d[:, 0:256], stage[:, 0:256])
            nc.scalar.dma_start(out_d[:, 256:512], stage[:, 256:512])

    nc.compile()
    return nc


_NC_CACHE = {}


def _get_nc():
    if "nc" not in _NC_CACHE:
        _NC_CACHE["nc"] = _build_nc()
    return _NC_CACHE["nc"]


def _in_maps(context, queries, Wc, Wq, bq, v):
    NS = len(SINES)
    NPROD = 2 * NS
    f16 = np.float16

    def fat(mat_t):            # [256 rows=d, X cols] -> [128, 2X]
        return np.concatenate([mat_t[0:128, :], mat_t[128:256, :]],
                              axis=1).astype(f16)

    wcT = fat(np.ascontiguousarray(Wc.T))
    wqT = fat(np.ascontiguousarray(Wq.T))
    # vm[:, 2*prod + et] = w_j * v[e-block et]
    vm = np.zeros((128, 2 * NPROD), dtype=np.float32)
    for j, (w, om) in enumerate(SINES):
        for col in (2 * j, 2 * j + 1):
            vm[:, 2 * col] = (np.float32(w) * v[0, 0:128]).astype(np.float32)
            vm[:, 2 * col + 1] = (np.float32(w) * v[0, 128:256]).astype(np.float32)
    bqr = bq.reshape(1, 256).astype(f16)
    maps = []
    for k in range(N_CORES):
        b = k // 4
        c0 = (k % 4) * CL
        qT = fat(np.ascontiguousarray(queries[b].T))
        ctxT = fat(np.ascontiguousarray(context[b, c0:c0 + CL, :].T))
        maps.append({"ww": np.ascontiguousarray(np.concatenate([wcT, wqT], axis=1)),
                     "aq": np.ascontiguousarray(np.concatenate([ctxT, qT], axis=1)),
                     "vm": vm, "bqr": bqr})
    return maps


def run(context, queries, Wc, Wq, bq, v, trace=False, **spmd_kwargs):
    nc = _get_nc()
    maps = _in_maps(np.asarray(context), np.asarray(queries), np.asarray(Wc),
                    np.asarray(Wq), np.asarray(bq), np.asarray(v))
    res = run_bass_kernel_spmd(nc, maps, core_ids=list(range(N_CORES)),
                               trace=trace, **spmd_kwargs)
    scores = np.empty((B, CTX, QRS), dtype=np.float32)
    for k in range(N_CORES):
        b = k // 4
        c0 = (k % 4) * CL
        arr = res.results[k]["out"].astype(np.float32)    # [128, 512]
        scores[b, c0:c0 + 128, :] = arr[:, 0:256]
        scores[b, c0 + 128:c0 + 256, :] = arr[:, 256:512]
    return scores.reshape(B, QRS, CTX), res


def kernel(context, queries, Wc, Wq, bq, v):
    out, _ = run(context, queries, Wc, Wq, bq, v, trace=False)
    return out
